# revision 15
# baseline (speedup 1.0000x reference)
"""Tensor-parallel Llama attention (decode, GQA, RoPE, KV-cache) on 8 TRN2 cores.

Sharding: core c owns kv-head c and q-heads 4c..4c+3. Wq/Wk/Wv are sharded
column-wise, Wo row-wise; each core computes a partial o_proj output and the
host sums the 8 partials (the all-reduce).

Per-core kernel layout notes:
  - Everything is kept "transposed" ([d, token] / [d, kpos]) so that every
    matmul contracts over the partition dim with M=128 (full PE array):
      qT/kT/vnew from projections, scoresT = kT_tile.T @ qT, attnT = v.T @ exp.
  - Softmax runs without max-subtraction (|score| <= ~8 here, exp is safe in
    fp32) so the kpos-partition layout only needs a sum: DVE accumulates exp
    tiles, a ones-column matmul reduces over partitions, and a 1x128 ones
    matmul broadcasts 1/denom back over partitions.
  - The causal mask only affects the 16 fresh keys (bottom-right aligned),
    applied as a 0/1 multiply on the one small fresh-score tile.
"""

import numpy as np

import concourse.bass as bass
import concourse.mybir as mybir
import concourse.tile as tile
from concourse import bacc
from concourse.bass_utils import run_bass_kernel_spmd

F32 = mybir.dt.float32
AF = mybir.ActivationFunctionType

# Problem shape (hardcoded per contract)
B, S, H = 4, 16, 4096
NH, NKV, HD = 32, 8, 128
PAST = 8192
ROPE_BASE = 10000.0
NCORES = 8
HQ = NH // NCORES          # q heads per core = 4
TOK = B * S                # 64 tokens
NCH = H // 128             # 32 contraction chunks for projections
ROWS = HQ * S              # 64 (head, token) query rows per batch
SCALE = HD ** -0.5


def build_nc(b=B, s=S, h=H, hq=HQ, hd=HD, past=PAST):
    tok = b * s
    nch = h // 128
    rows = hq * s
    ktiles = past // 128
    halves = 2                      # stream k/v caches in 2 chunks per batch
    kt_half = ktiles // halves

    nc = bacc.Bacc("TRN2", target_bir_lowering=False, debug=False)

    hiddenT_d = nc.dram_tensor("hiddenT", [h, tok], F32, kind="ExternalInput").ap()
    wq_d = nc.dram_tensor("wq", [h, hq * hd], F32, kind="ExternalInput").ap()
    wkv_d = nc.dram_tensor("wkv", [h, 2 * hd], F32, kind="ExternalInput").ap()
    wo_d = nc.dram_tensor("wo", [hq * hd, h], F32, kind="ExternalInput").ap()
    kT_d = nc.dram_tensor("kT", [b, hd, past], F32, kind="ExternalInput").ap()
    v_d = nc.dram_tensor("v", [b, past, hd], F32, kind="ExternalInput").ap()
    cosT_d = nc.dram_tensor("cosT", [hd, tok], F32, kind="ExternalInput").ap()
    sinT_d = nc.dram_tensor("sinT", [hd, tok], F32, kind="ExternalInput").ap()
    nsinT_d = nc.dram_tensor("nsinT", [hd, tok], F32, kind="ExternalInput").ap()
    maskT_d = nc.dram_tensor("maskT", [s, rows], F32, kind="ExternalInput").ap()
    out_d = nc.dram_tensor("out_p", [tok, h], F32, kind="ExternalOutput").ap()

    with tile.TileContext(nc) as tc:
        import contextlib

        with contextlib.ExitStack() as ctx:
            ep = ctx.enter_context          # shorthand
            const_p = ep(tc.tile_pool(name="const", bufs=1))
            hT_p = ep(tc.tile_pool(name="hT", bufs=1))
            wq_p = ep(tc.tile_pool(name="wq", bufs=3))
            wkv_p = ep(tc.tile_pool(name="wkv", bufs=3))
            wo_p = ep(tc.tile_pool(name="wo", bufs=6))
            kv_p = ep(tc.tile_pool(name="kv", bufs=3))
            qkv_p = ep(tc.tile_pool(name="qkv", bufs=1))
            rope_p = ep(tc.tile_pool(name="rope", bufs=4))
            exp_p = ep(tc.tile_pool(name="exp", bufs=6))
            acc_p = ep(tc.tile_pool(name="acc", bufs=2))
            den_p = ep(tc.tile_pool(name="den", bufs=2))
            # PSUM: 8 banks total; tags share banks across phases:
            #   "A"(2): qt (proj) -> ops (o_proj);  "attn"(2): per-batch attn acc
            #   "B"(2): ktn+vn (proj) -> dsum/bc (softmax);  "sc"(2): score tiles
            ps = ep(tc.tile_pool(name="ps", bufs=2, space="PSUM"))

            # ---- constants ----
            ones_col = const_p.tile([128, 1], F32)
            nc.vector.memset(ones_col[:], 1.0)
            ones_row = const_p.tile([1, 128], F32)
            nc.vector.memset(ones_row[:], 1.0)
            cosT = const_p.tile([hd, tok], F32)
            nc.sync.dma_start(cosT[:], cosT_d[:])
            sinT = const_p.tile([hd, tok], F32)
            nc.sync.dma_start(sinT[:], sinT_d[:])
            nsinT = const_p.tile([hd, tok], F32)
            nc.sync.dma_start(nsinT[:], nsinT_d[:])
            maskT = const_p.tile([s, rows], F32)
            nc.sync.dma_start(maskT[:], maskT_d[:])
            ident = const_p.tile([tok, tok], F32)
            from concourse.masks import make_identity
            make_identity(nc, ident[:])

            # ---- load hiddenT: [h, tok] -> sbuf [128, nch*tok] ----
            hT = hT_p.tile([128, nch * tok], F32)
            nc.sync.dma_start(
                hT[:].rearrange("p (c t) -> p c t", c=nch),
                hiddenT_d.rearrange("(c p) t -> p c t", p=128),
            )

            # ---- projections: qT_ps[j] [128, tok], kT_ps [128, tok], v_ps [tok, 128] ----
            # q in token-major [tok, hq*hd] (single PSUM bank/group); k/v direct
            q_ps = ps.tile([tok, hq * hd], F32, tag="A")
            kT_ps = ps.tile([128, tok], F32, tag="B")
            v_ps = ps.tile([tok, 128], F32, tag="B")
            for c in range(nch):
                wq_t = wq_p.tile([128, hq * hd], F32)
                nc.sync.dma_start(
                    wq_t[:], wq_d.rearrange("(c p) m -> c p m", p=128)[c]
                )
                wkv_t = wkv_p.tile([128, 2 * hd], F32)
                nc.sync.dma_start(
                    wkv_t[:], wkv_d.rearrange("(c p) m -> c p m", p=128)[c]
                )
                rhs_h = hT[:, c * tok:(c + 1) * tok]
                fl = dict(start=(c == 0), stop=(c == nch - 1))
                nc.tensor.matmul(q_ps[:], rhs_h, wq_t[:], **fl)
                nc.tensor.matmul(kT_ps[:], wkv_t[:, 0:hd], rhs_h, **fl)
                nc.tensor.matmul(v_ps[:], rhs_h, wkv_t[:, hd:2 * hd], **fl)
            q_sb = qkv_p.tile([tok, hq * hd], F32, tag="qsb")
            nc.scalar.copy(q_sb[:], q_ps[:])

            # ---- RoPE -> qT_sb [128, (b,hq,s)], kT_new [128, (b,s)], v_new [tok, 128] ----
            half = hd // 2
            qT_sb = qkv_p.tile([128, b * rows], F32, tag="qT")
            kT_new = qkv_p.tile([128, tok], F32, tag="kTn")
            # per-batch fresh-v tiles at base partition 0 (PE wants base 0/32/64)
            v_new = [
                qkv_p.tile([s, hd], F32, tag=f"vnew{bb}", name=f"vnew{bb}")
                for bb in range(b)
            ]

            def rope(dst, src_ps):
                # dst = src*cos + rotate_half(src)*sin  (all [128, tok], (b,t) cols)
                t1 = rope_p.tile([128, tok], F32, tag="r1")
                nc.vector.tensor_mul(t1[:], src_ps[:], cosT[:])
                t2 = rope_p.tile([128, tok], F32, tag="r2")
                nc.vector.tensor_mul(
                    t2[0:half, :], src_ps[half:hd, :], nsinT[0:half, :]
                )
                nc.vector.tensor_mul(
                    t2[half:hd, :], src_ps[0:half, :], sinT[half:hd, :]
                )
                nc.vector.tensor_add(dst, t1[:], t2[:])
                return dst

            for j in range(hq):
                # transpose head j to [d, (b,t)], then rope-scatter to (b, j, t)
                qt_ps = ps.tile([hd, tok], F32, tag="sc", name=f"qtp{j}")
                nc.tensor.transpose(
                    qt_ps[:], q_sb[:, j * hd:(j + 1) * hd], ident[:]
                )
                dst = qT_sb[:].rearrange("p (bb j t) -> p bb j t", bb=b, j=hq)[:, :, j, :]
                rope(dst, qt_ps)
            rope(kT_new[:], kT_ps)
            v_sb = qkv_p.tile([tok, hd], F32, tag="vsb")
            nc.scalar.copy(v_sb[:], v_ps[:])
            for bb in range(b):
                nc.sync.dma_start(v_new[bb][:], v_sb[bb * s:(bb + 1) * s, :])

            # ---- attention per batch ----
            attnT_sb = qkv_p.tile([128, hq * tok], F32, tag="attnT")  # (h, b, t) cols
            for bb in range(b):
                qT_b = qT_sb[:, bb * rows:(bb + 1) * rows]  # [128, (h,t)]
                attn_ps = ps.tile([128, rows], F32, tag="attn")
                acc = acc_p.tile([128, rows], F32, tag="acc")
                for hf in range(halves):
                    kt = kv_p.tile([128, kt_half * 128], F32, tag="kt")
                    nc.sync.dma_start(
                        kt[:], kT_d[bb, :, hf * kt_half * 128:(hf + 1) * kt_half * 128]
                    )
                    vt = kv_p.tile([128, kt_half * hd], F32, tag="vt")
                    nc.sync.dma_start(
                        vt[:].rearrange("p (tt d) -> p tt d", tt=kt_half),
                        v_d[bb].rearrange("(tt p) d -> p tt d", p=128)[
                            :, hf * kt_half:(hf + 1) * kt_half, :
                        ],
                    )
                    for tt in range(kt_half):
                        t = hf * kt_half + tt
                        sc_ps = ps.tile([128, rows], F32, tag="sc")
                        nc.tensor.matmul(
                            sc_ps[:], kt[:, tt * 128:(tt + 1) * 128], qT_b,
                            start=True, stop=True,
                        )
                        ex = exp_p.tile([128, rows], F32, tag="ex")
                        nc.scalar.activation(ex[:], sc_ps[:], AF.Exp)
                        if t == 0:
                            nc.vector.tensor_copy(acc[:], ex[:])
                        else:
                            nc.vector.tensor_add(acc[:], acc[:], ex[:])
                        nc.tensor.matmul(
                            attn_ps[:], vt[:, tt * hd:(tt + 1) * hd], ex[:],
                            start=(t == 0), stop=False, skip_group_check=True,
                        )
                # fresh keys (the only masked block)
                scn_ps = ps.tile([s, rows], F32, tag="sc")
                nc.tensor.matmul(
                    scn_ps[:], kT_new[:, bb * s:(bb + 1) * s], qT_b,
                    start=True, stop=True,
                )
                exn = exp_p.tile([s, rows], F32, tag="exn")
                nc.scalar.activation(exn[:], scn_ps[:], AF.Exp)
                nc.vector.tensor_mul(exn[:], exn[:], maskT[:])
                nc.vector.tensor_add(acc[0:s, :], acc[0:s, :], exn[:])
                nc.tensor.matmul(
                    attn_ps[:], v_new[bb][:], exn[:],
                    start=False, stop=True, skip_group_check=True,
                )
                # denominator: reduce acc over partitions, broadcast reciprocal
                dsum_ps = ps.tile([1, rows], F32, tag="B")
                nc.tensor.matmul(dsum_ps[:], ones_col[:], acc[:], start=True, stop=True)
                rden = den_p.tile([1, rows], F32, tag="rden")
                nc.vector.reciprocal(rden[:], dsum_ps[:])
                bc_ps = ps.tile([128, rows], F32, tag="B")
                nc.tensor.matmul(bc_ps[:], ones_row[:], rden[:], start=True, stop=True)
                rdenb = den_p.tile([128, rows], F32, tag="rdenb")
                nc.scalar.copy(rdenb[:], bc_ps[:])
                # normalize + scatter (h,t) -> (h, b, t)
                dst = attnT_sb[:].rearrange("p (j bb t) -> p j bb t", j=hq, bb=b)[
                    :, :, bb, :
                ]
                nc.vector.tensor_mul(
                    dst,
                    attn_ps[:].rearrange("p (j t) -> p j t", j=hq),
                    rdenb[:].rearrange("p (j t) -> p j t", j=hq),
                )

            # ---- o_proj: out[tok, h] = sum_j attnT_j.T @ wo_j ----
            for nt in range(h // 512):
                o_ps = ps.tile([tok, 512], F32, tag="A")
                for j in range(hq):
                    wo_t = wo_p.tile([128, 512], F32, tag="wo")
                    nc.sync.dma_start(
                        wo_t[:],
                        wo_d.rearrange("(j p) m -> j p m", p=128)[
                            j, :, nt * 512:(nt + 1) * 512
                        ],
                    )
                    nc.tensor.matmul(
                        o_ps[:], attnT_sb[:, j * tok:(j + 1) * tok], wo_t[:],
                        start=(j == 0), stop=(j == hq - 1),
                    )
                o_sb = wo_p.tile([tok, 512], F32, tag="osb", bufs=3)
                nc.scalar.copy(o_sb[:], o_ps[:])
                nc.sync.dma_start(out_d[:, nt * 512:(nt + 1) * 512], o_sb[:])

    nc.compile()
    return nc


_NC_CACHE = {}


def _get_nc(key=(B, S, H, HQ, HD, PAST)):
    if key not in _NC_CACHE:
        _NC_CACHE[key] = build_nc(*key)
    return _NC_CACHE[key]


def make_in_maps(hidden_states, k_cache, v_cache, Wq, Wk, Wv, Wo, position_ids):
    """Host-side shard + layout prep: one input dict per core."""
    hiddenT = np.ascontiguousarray(
        hidden_states.reshape(TOK, H).T, dtype=np.float32
    )
    # RoPE tables in [d, (b, t)] layout, duplicated freq block (half-split rope)
    inv_freq = (1.0 / (ROPE_BASE ** (np.arange(0, HD, 2, dtype=np.float64) / HD)))
    ang = position_ids.astype(np.float64).reshape(-1)[None, :] * np.concatenate(
        [inv_freq, inv_freq]
    )[:, None]                                           # [hd, tok]
    cosT = np.cos(ang).astype(np.float32)
    sinT = np.sin(ang).astype(np.float32)
    nsinT = (-sinT).copy()
    # mask over fresh keys: maskT[j, (h, t)] = 1 if j <= t (bottom-right causal)
    jj = np.arange(S)[:, None]
    tt = np.tile(np.arange(S)[None, :], (1, HQ)).reshape(1, ROWS)
    maskT = (jj <= tt).astype(np.float32)

    in_maps = []
    for c in range(NCORES):
        q0 = c * HQ * HD
        in_maps.append({
            "hiddenT": hiddenT,
            "wq": np.ascontiguousarray(Wq[:, q0:q0 + HQ * HD] * SCALE,
                                       dtype=np.float32),
            "wkv": np.ascontiguousarray(
                np.concatenate(
                    [Wk[:, c * HD:(c + 1) * HD], Wv[:, c * HD:(c + 1) * HD]], axis=1
                ), dtype=np.float32),
            "wo": np.ascontiguousarray(Wo[q0:q0 + HQ * HD, :], dtype=np.float32),
            "kT": np.ascontiguousarray(
                k_cache[:, :, c, :].transpose(0, 2, 1), dtype=np.float32),
            "v": np.ascontiguousarray(v_cache[:, :, c, :], dtype=np.float32),
            "cosT": cosT, "sinT": sinT, "nsinT": nsinT, "maskT": maskT,
        })
    return in_maps


def kernel(hidden_states, k_cache, v_cache, Wq, Wk, Wv, Wo, position_ids):
    hidden_states = np.asarray(hidden_states)
    nc = _get_nc()
    in_maps = make_in_maps(
        np.asarray(hidden_states), np.asarray(k_cache), np.asarray(v_cache),
        np.asarray(Wq), np.asarray(Wk), np.asarray(Wv), np.asarray(Wo),
        np.asarray(position_ids),
    )
    res = run_bass_kernel_spmd(nc, in_maps, list(range(NCORES)))
    out = np.zeros((TOK, H), np.float32)
    for c in range(NCORES):
        out += res.results[c]["out_p"]
    return out.reshape(B, S, H)


# revision 22
# speedup vs baseline: 1.7100x; 1.7100x over previous
"""Tensor-parallel Llama attention (decode, GQA, RoPE, KV-cache) on 8 TRN2 cores.

Sharding: core c owns kv-head c and q-heads 4c..4c+3. Wq/Wk/Wv are sharded
column-wise, Wo row-wise; each core computes a partial o_proj output and the
host sums the 8 partials (the all-reduce).

Per-core kernel layout notes:
  - Everything is kept "transposed" ([d, token] / [d, kpos]) so that every
    matmul contracts over the partition dim with M=128 (full PE array):
      qT/kT/vnew from projections, scoresT = kT_tile.T @ qT, attnT = v.T @ exp.
  - Softmax runs without max-subtraction (|score| <= ~8 here, exp is safe in
    fp32) so the kpos-partition layout only needs a sum: DVE accumulates exp
    tiles, a ones-column matmul reduces over partitions, and a 1x128 ones
    matmul broadcasts 1/denom back over partitions.
  - The causal mask only affects the 16 fresh keys (bottom-right aligned),
    applied as a 0/1 multiply on the one small fresh-score tile.
"""

import numpy as np
import ml_dtypes

import concourse.bass as bass
import concourse.mybir as mybir
import concourse.tile as tile
from concourse import bacc
from concourse.bass_utils import run_bass_kernel_spmd

F32 = mybir.dt.float32
BF16 = mybir.dt.bfloat16
AF = mybir.ActivationFunctionType

# Problem shape (hardcoded per contract)
B, S, H = 4, 16, 4096
NH, NKV, HD = 32, 8, 128
PAST = 8192
ROPE_BASE = 10000.0
NCORES = 8
HQ = NH // NCORES          # q heads per core = 4
TOK = B * S                # 64 tokens
NCH = H // 128             # 32 contraction chunks for projections
ROWS = HQ * S              # 64 (head, token) query rows per batch
SCALE = HD ** -0.5


def build_nc(b=B, s=S, h=H, hq=HQ, hd=HD, past=PAST):
    tok = b * s
    nch = h // 128
    rows = hq * s
    ktiles = past // 128
    halves = 2                      # stream k/v caches in 2 chunks per batch
    kt_half = ktiles // halves

    nc = bacc.Bacc("TRN2", target_bir_lowering=False, debug=False)

    hiddenT_d = nc.dram_tensor("hiddenT", [h, tok], F32, kind="ExternalInput").ap()
    wq_d = nc.dram_tensor("wq", [h, hq * hd], F32, kind="ExternalInput").ap()
    wkv_d = nc.dram_tensor("wkv", [h, 2 * hd], F32, kind="ExternalInput").ap()
    wo_d = nc.dram_tensor("wo", [hq * hd, h], F32, kind="ExternalInput").ap()
    kT_d = nc.dram_tensor("kT", [b, hd, past], BF16, kind="ExternalInput").ap()
    v_d = nc.dram_tensor("v", [b, 128, past], BF16, kind="ExternalInput").ap()
    cosT_d = nc.dram_tensor("cosT", [hd, tok], F32, kind="ExternalInput").ap()
    sinT_d = nc.dram_tensor("sinT", [hd, tok], F32, kind="ExternalInput").ap()
    nsinT_d = nc.dram_tensor("nsinT", [hd, tok], F32, kind="ExternalInput").ap()
    maskT_d = nc.dram_tensor("maskT", [s, rows], F32, kind="ExternalInput").ap()
    out_d = nc.dram_tensor("out_p", [tok, h], F32, kind="ExternalOutput").ap()

    with tile.TileContext(nc) as tc:
        import contextlib

        with contextlib.ExitStack() as ctx:
            ep = ctx.enter_context          # shorthand
            const_p = ep(tc.tile_pool(name="const", bufs=1))
            hT_p = ep(tc.tile_pool(name="hT", bufs=1))
            wq_p = ep(tc.tile_pool(name="wq", bufs=3))
            wkv_p = ep(tc.tile_pool(name="wkv", bufs=3))
            wo_p = ep(tc.tile_pool(name="wo", bufs=14))
            kv_p = ep(tc.tile_pool(name="kv", bufs=3))
            qkv_p = ep(tc.tile_pool(name="qkv", bufs=1))
            rope_p = ep(tc.tile_pool(name="rope", bufs=4))
            exp_p = ep(tc.tile_pool(name="exp", bufs=4))
            acc_p = ep(tc.tile_pool(name="acc", bufs=2))
            den_p = ep(tc.tile_pool(name="den", bufs=2))
            # PSUM: 8 banks total; tags share banks across phases:
            #   "A"(2): qt (proj) -> ops (o_proj);  "attn"(2): per-batch attn acc
            #   "B"(2): ktn+vn (proj) -> dsum/bc (softmax);  "sc"(2): score tiles
            ps = ep(tc.tile_pool(name="ps", bufs=2, space="PSUM"))

            # ---- constants ----
            ones_col = const_p.tile([128, 1], F32)
            nc.vector.memset(ones_col[:], 1.0)
            ones_row = const_p.tile([1, 128], F32)
            nc.vector.memset(ones_row[:], 1.0)
            cosT = const_p.tile([hd, tok], F32)
            nc.sync.dma_start(cosT[:], cosT_d[:])
            sinT = const_p.tile([hd, tok], F32)
            nc.sync.dma_start(sinT[:], sinT_d[:])
            nsinT = const_p.tile([hd, tok], F32)
            nc.sync.dma_start(nsinT[:], nsinT_d[:])
            maskT = const_p.tile([s, rows], F32)
            nc.sync.dma_start(maskT[:], maskT_d[:])
            ident = const_p.tile([tok, tok], F32)
            from concourse.masks import make_identity
            make_identity(nc, ident[:])

            # ---- load hiddenT: [h, tok] -> sbuf [128, nch*tok] ----
            hT = hT_p.tile([128, nch * tok], F32)
            nc.sync.dma_start(
                hT[:].rearrange("p (c t) -> p c t", c=nch),
                hiddenT_d.rearrange("(c p) t -> p c t", p=128),
            )

            # ---- projections: qT_ps[j] [128, tok], kT_ps [128, tok], v_ps [tok, 128] ----
            # q in token-major [tok, hq*hd] (single PSUM bank/group); k/v direct
            q_ps = ps.tile([tok, hq * hd], F32, tag="A")
            kT_ps = ps.tile([128, tok], F32, tag="B")
            v_ps = ps.tile([tok, 128], F32, tag="B")
            for c in range(nch):
                wq_t = wq_p.tile([128, hq * hd], F32)
                nc.sync.dma_start(
                    wq_t[:], wq_d.rearrange("(c p) m -> c p m", p=128)[c]
                )
                wkv_t = wkv_p.tile([128, 2 * hd], F32)
                nc.sync.dma_start(
                    wkv_t[:], wkv_d.rearrange("(c p) m -> c p m", p=128)[c]
                )
                rhs_h = hT[:, c * tok:(c + 1) * tok]
                fl = dict(start=(c == 0), stop=(c == nch - 1))
                nc.tensor.matmul(q_ps[:], rhs_h, wq_t[:], **fl)
                nc.tensor.matmul(kT_ps[:], wkv_t[:, 0:hd], rhs_h, **fl)
                nc.tensor.matmul(v_ps[:], rhs_h, wkv_t[:, hd:2 * hd], **fl)
            q_sb = qkv_p.tile([tok, hq * hd], F32, tag="qsb")
            nc.scalar.copy(q_sb[:], q_ps[:])

            # ---- RoPE -> qT_sb [128, (b,hq,s)], kT_new [128, (b,s)], v_new [tok, 128] ----
            half = hd // 2
            qT_sb = qkv_p.tile([128, b * rows], F32, tag="qT")
            kT_new = qkv_p.tile([128, tok], F32, tag="kTn")
            # per-batch fresh-v tiles at base partition 0 (PE wants base 0/32/64)
            v_new = [
                qkv_p.tile([s, hd], F32, tag=f"vnew{bb}", name=f"vnew{bb}")
                for bb in range(b)
            ]

            def rope(dst, src_ps):
                # dst = src*cos + rotate_half(src)*sin  (all [128, tok], (b,t) cols)
                t1 = rope_p.tile([128, tok], F32, tag="r1")
                nc.vector.tensor_mul(t1[:], src_ps[:], cosT[:])
                t2 = rope_p.tile([128, tok], F32, tag="r2")
                nc.vector.tensor_mul(
                    t2[0:half, :], src_ps[half:hd, :], nsinT[0:half, :]
                )
                nc.vector.tensor_mul(
                    t2[half:hd, :], src_ps[0:half, :], sinT[half:hd, :]
                )
                nc.vector.tensor_add(dst, t1[:], t2[:])
                return dst

            for j in range(hq):
                # transpose head j to [d, (b,t)], then rope-scatter to (b, j, t)
                qt_ps = ps.tile([hd, tok], F32, tag="sc", name=f"qtp{j}")
                nc.tensor.transpose(
                    qt_ps[:], q_sb[:, j * hd:(j + 1) * hd], ident[:]
                )
                dst = qT_sb[:].rearrange("p (bb j t) -> p bb j t", bb=b, j=hq)[:, :, j, :]
                rope(dst, qt_ps)
            rope(kT_new[:], kT_ps)
            v_sb = qkv_p.tile([tok, hd], F32, tag="vsb")
            nc.scalar.copy(v_sb[:], v_ps[:])
            for bb in range(b):
                nc.sync.dma_start(v_new[bb][:], v_sb[bb * s:(bb + 1) * s, :])

            qT_bf = qkv_p.tile([128, b * rows], BF16, tag="qTbf")
            nc.vector.tensor_copy(qT_bf[:], qT_sb[:])

            # ---- attention per batch ----
            # Scores are built 8 kpos-tiles at a time into ONE psum bank
            # (disjoint column ranges, one accumulation group) so exp / the
            # denominator reduce run 512 wide, 8x fewer cross-engine hops.
            GRP = 512 // rows               # kpos tiles per score group (8)
            attnT_sb = qkv_p.tile([128, hq * tok], F32, tag="attnT")  # (h, b, t) cols
            for bb in range(b):
                qT_b = qT_bf[:, bb * rows:(bb + 1) * rows]  # [128, (h,t)] bf16
                qT_b32 = qT_sb[:, bb * rows:(bb + 1) * rows]
                attn_ps = ps.tile([128, rows], F32, tag="attn")
                acc = acc_p.tile([128, rows], F32, tag="acc")
                for hf in range(halves):
                    kt = kv_p.tile([128, kt_half * 128], BF16, tag="kt")
                    nc.sync.dma_start(
                        kt[:], kT_d[bb, :, hf * kt_half * 128:(hf + 1) * kt_half * 128]
                    )
                    vt = kv_p.tile([128, kt_half * hd], BF16, tag="vt")
                    nc.sync.dma_start(
                        vt[:],
                        v_d[bb, :, hf * kt_half * hd:(hf + 1) * kt_half * hd],
                    )
                    for g in range(kt_half // GRP):
                        sc_ps = ps.tile([128, GRP * rows], F32, tag="sc")
                        for u in range(GRP):
                            tt = g * GRP + u
                            nc.tensor.matmul(
                                sc_ps[:, u * rows:(u + 1) * rows],
                                kt[:, tt * 128:(tt + 1) * 128], qT_b,
                                start=(u == 0), stop=(u == GRP - 1),
                            )
                        ex = exp_p.tile([128, GRP * rows], BF16, tag="ex")
                        nc.scalar.activation(ex[:], sc_ps[:], AF.Exp)
                        red = acc if (hf == 0 and g == 0) else acc_p.tile(
                            [128, rows], F32, tag="red", name="red")
                        nc.vector.tensor_reduce(
                            red[:],
                            ex[:].rearrange("p (u q) -> p q u", u=GRP),
                            axis=mybir.AxisListType.X, op=mybir.AluOpType.add,
                        )
                        if red is not acc:
                            nc.vector.tensor_add(acc[:], acc[:], red[:])
                        for u in range(GRP):
                            tt = g * GRP + u
                            t = hf * kt_half + tt
                            nc.tensor.matmul(
                                attn_ps[:], vt[:, tt * hd:(tt + 1) * hd],
                                ex[:, u * rows:(u + 1) * rows],
                                start=(t == 0), stop=False, skip_group_check=True,
                            )
                # fresh keys (the only masked block)
                scn_ps = ps.tile([s, rows], F32, tag="sc")
                nc.tensor.matmul(
                    scn_ps[:], kT_new[:, bb * s:(bb + 1) * s], qT_b32,
                    start=True, stop=True,
                )
                exn = exp_p.tile([s, rows], F32, tag="exn")
                nc.scalar.activation(exn[:], scn_ps[:], AF.Exp)
                nc.vector.tensor_mul(exn[:], exn[:], maskT[:])
                nc.vector.tensor_add(acc[0:s, :], acc[0:s, :], exn[:])
                nc.tensor.matmul(
                    attn_ps[:], v_new[bb][:], exn[:],
                    start=False, stop=True, skip_group_check=True,
                )
                # denominator: reduce acc over partitions, broadcast reciprocal
                dsum_ps = ps.tile([1, rows], F32, tag="B")
                nc.tensor.matmul(dsum_ps[:], ones_col[:], acc[:], start=True, stop=True)
                rden = den_p.tile([1, rows], F32, tag="rden")
                nc.vector.reciprocal(rden[:], dsum_ps[:])
                bc_ps = ps.tile([128, rows], F32, tag="B")
                nc.tensor.matmul(bc_ps[:], ones_row[:], rden[:], start=True, stop=True)
                rdenb = den_p.tile([128, rows], F32, tag="rdenb")
                nc.scalar.copy(rdenb[:], bc_ps[:])
                # normalize + scatter (h,t) -> (h, b, t)
                dst = attnT_sb[:].rearrange("p (j bb t) -> p j bb t", j=hq, bb=b)[
                    :, :, bb, :
                ]
                nc.vector.tensor_mul(
                    dst,
                    attn_ps[:].rearrange("p (j t) -> p j t", j=hq),
                    rdenb[:].rearrange("p (j t) -> p j t", j=hq),
                )

            # ---- o_proj: out[tok, h] = sum_j attnT_j.T @ wo_j ----
            for nt in range(h // 512):
                o_ps = ps.tile([tok, 512], F32, tag="A")
                for j in range(hq):
                    wo_t = wo_p.tile([128, 512], F32, tag="wo")
                    nc.sync.dma_start(
                        wo_t[:],
                        wo_d.rearrange("(j p) m -> j p m", p=128)[
                            j, :, nt * 512:(nt + 1) * 512
                        ],
                    )
                    nc.tensor.matmul(
                        o_ps[:], attnT_sb[:, j * tok:(j + 1) * tok], wo_t[:],
                        start=(j == 0), stop=(j == hq - 1),
                    )
                o_sb = wo_p.tile([tok, 512], F32, tag="osb", bufs=3)
                nc.scalar.copy(o_sb[:], o_ps[:])
                nc.sync.dma_start(out_d[:, nt * 512:(nt + 1) * 512], o_sb[:])

    nc.compile()
    return nc


_NC_CACHE = {}


def _get_nc(key=(B, S, H, HQ, HD, PAST)):
    if key not in _NC_CACHE:
        _NC_CACHE[key] = build_nc(*key)
    return _NC_CACHE[key]


def make_in_maps(hidden_states, k_cache, v_cache, Wq, Wk, Wv, Wo, position_ids):
    """Host-side shard + layout prep: one input dict per core."""
    hiddenT = np.ascontiguousarray(
        hidden_states.reshape(TOK, H).T, dtype=np.float32
    )
    # RoPE tables in [d, (b, t)] layout, duplicated freq block (half-split rope)
    inv_freq = (1.0 / (ROPE_BASE ** (np.arange(0, HD, 2, dtype=np.float64) / HD)))
    ang = position_ids.astype(np.float64).reshape(-1)[None, :] * np.concatenate(
        [inv_freq, inv_freq]
    )[:, None]                                           # [hd, tok]
    cosT = np.cos(ang).astype(np.float32)
    sinT = np.sin(ang).astype(np.float32)
    nsinT = (-sinT).copy()
    # mask over fresh keys: maskT[j, (h, t)] = 1 if j <= t (bottom-right causal)
    jj = np.arange(S)[:, None]
    tt = np.tile(np.arange(S)[None, :], (1, HQ)).reshape(1, ROWS)
    maskT = (jj <= tt).astype(np.float32)

    in_maps = []
    for c in range(NCORES):
        q0 = c * HQ * HD
        in_maps.append({
            "hiddenT": hiddenT,
            "wq": np.ascontiguousarray(Wq[:, q0:q0 + HQ * HD] * SCALE,
                                       dtype=np.float32),
            "wkv": np.ascontiguousarray(
                np.concatenate(
                    [Wk[:, c * HD:(c + 1) * HD], Wv[:, c * HD:(c + 1) * HD]], axis=1
                ), dtype=np.float32),
            "wo": np.ascontiguousarray(Wo[q0:q0 + HQ * HD, :], dtype=np.float32),
            "kT": np.ascontiguousarray(
                k_cache[:, :, c, :].transpose(0, 2, 1)).astype(ml_dtypes.bfloat16),
            # pre-permuted to the sbuf tile layout: v_r[b, p, tt*HD+d] =
            # v[b, tt*128+p, d] -> fully contiguous 8KB DMA rows
            "v": np.ascontiguousarray(
                v_cache[:, :, c, :].reshape(B, PAST // 128, 128, HD)
                .transpose(0, 2, 1, 3).reshape(B, 128, PAST)
            ).astype(ml_dtypes.bfloat16),
            "cosT": cosT, "sinT": sinT, "nsinT": nsinT, "maskT": maskT,
        })
    return in_maps


def kernel(hidden_states, k_cache, v_cache, Wq, Wk, Wv, Wo, position_ids):
    hidden_states = np.asarray(hidden_states)
    nc = _get_nc()
    in_maps = make_in_maps(
        np.asarray(hidden_states), np.asarray(k_cache), np.asarray(v_cache),
        np.asarray(Wq), np.asarray(Wk), np.asarray(Wv), np.asarray(Wo),
        np.asarray(position_ids),
    )
    res = run_bass_kernel_spmd(nc, in_maps, list(range(NCORES)))
    out = np.zeros((TOK, H), np.float32)
    for c in range(NCORES):
        out += res.results[c]["out_p"]
    return out.reshape(B, S, H)


# revision 23
# speedup vs baseline: 1.9509x; 1.1409x over previous
"""Tensor-parallel Llama attention (decode, GQA, RoPE, KV-cache) on 8 TRN2 cores.

Sharding: core c owns kv-head c and q-heads 4c..4c+3. Wq/Wk/Wv are sharded
column-wise, Wo row-wise; each core computes a partial o_proj output and the
host sums the 8 partials (the all-reduce).

Per-core kernel layout notes:
  - Everything is kept "transposed" ([d, token] / [d, kpos]) so that every
    matmul contracts over the partition dim with M=128 (full PE array):
      qT/kT/vnew from projections, scoresT = kT_tile.T @ qT, attnT = v.T @ exp.
  - Softmax runs without max-subtraction (|score| <= ~8 here, exp is safe in
    fp32) so the kpos-partition layout only needs a sum: DVE accumulates exp
    tiles, a ones-column matmul reduces over partitions, and a 1x128 ones
    matmul broadcasts 1/denom back over partitions.
  - The causal mask only affects the 16 fresh keys (bottom-right aligned),
    applied as a 0/1 multiply on the one small fresh-score tile.
"""

import numpy as np
import ml_dtypes

import concourse.bass as bass
import concourse.mybir as mybir
import concourse.tile as tile
from concourse import bacc
from concourse.bass_utils import run_bass_kernel_spmd

F32 = mybir.dt.float32
BF16 = mybir.dt.bfloat16
AF = mybir.ActivationFunctionType

# Problem shape (hardcoded per contract)
B, S, H = 4, 16, 4096
NH, NKV, HD = 32, 8, 128
PAST = 8192
ROPE_BASE = 10000.0
NCORES = 8
HQ = NH // NCORES          # q heads per core = 4
TOK = B * S                # 64 tokens
NCH = H // 128             # 32 contraction chunks for projections
ROWS = HQ * S              # 64 (head, token) query rows per batch
SCALE = HD ** -0.5


def build_nc(b=B, s=S, h=H, hq=HQ, hd=HD, past=PAST):
    tok = b * s
    nch = h // 128
    rows = hq * s
    ktiles = past // 128
    halves = 2                      # stream k/v caches in 2 chunks per batch
    kt_half = ktiles // halves

    nc = bacc.Bacc("TRN2", target_bir_lowering=False, debug=False)

    hiddenT_d = nc.dram_tensor("hiddenT", [h, tok], BF16, kind="ExternalInput").ap()
    wq_d = nc.dram_tensor("wq", [h, hq * hd], BF16, kind="ExternalInput").ap()
    wkv_d = nc.dram_tensor("wkv", [h, 2 * hd], BF16, kind="ExternalInput").ap()
    wo_d = nc.dram_tensor("wo", [hq * hd, h], BF16, kind="ExternalInput").ap()
    kT_d = nc.dram_tensor("kT", [b, hd, past], BF16, kind="ExternalInput").ap()
    v_d = nc.dram_tensor("v", [b, 128, past], BF16, kind="ExternalInput").ap()
    cosT_d = nc.dram_tensor("cosT", [hd, tok], F32, kind="ExternalInput").ap()
    sinT_d = nc.dram_tensor("sinT", [hd, tok], F32, kind="ExternalInput").ap()
    nsinT_d = nc.dram_tensor("nsinT", [hd, tok], F32, kind="ExternalInput").ap()
    maskT_d = nc.dram_tensor("maskT", [s, rows], F32, kind="ExternalInput").ap()
    out_d = nc.dram_tensor("out_p", [tok, h], F32, kind="ExternalOutput").ap()

    with tile.TileContext(nc) as tc:
        import contextlib

        with contextlib.ExitStack() as ctx:
            ep = ctx.enter_context          # shorthand
            const_p = ep(tc.tile_pool(name="const", bufs=1))
            hT_p = ep(tc.tile_pool(name="hT", bufs=1))
            wq_p = ep(tc.tile_pool(name="wq", bufs=3))
            wkv_p = ep(tc.tile_pool(name="wkv", bufs=3))
            wo_p = ep(tc.tile_pool(name="wo", bufs=14))
            kv_p = ep(tc.tile_pool(name="kv", bufs=3))
            qkv_p = ep(tc.tile_pool(name="qkv", bufs=1))
            rope_p = ep(tc.tile_pool(name="rope", bufs=4))
            exp_p = ep(tc.tile_pool(name="exp", bufs=4))
            acc_p = ep(tc.tile_pool(name="acc", bufs=2))
            den_p = ep(tc.tile_pool(name="den", bufs=2))
            # PSUM: 8 banks total; tags share banks across phases:
            #   "A"(2): qt (proj) -> ops (o_proj);  "attn"(2): per-batch attn acc
            #   "B"(2): ktn+vn (proj) -> dsum/bc (softmax);  "sc"(2): score tiles
            ps = ep(tc.tile_pool(name="ps", bufs=2, space="PSUM"))

            # ---- constants ----
            ones_col = const_p.tile([128, 1], F32)
            nc.vector.memset(ones_col[:], 1.0)
            ones_row = const_p.tile([1, 128], F32)
            nc.vector.memset(ones_row[:], 1.0)
            cosT = const_p.tile([hd, tok], F32)
            nc.sync.dma_start(cosT[:], cosT_d[:])
            sinT = const_p.tile([hd, tok], F32)
            nc.sync.dma_start(sinT[:], sinT_d[:])
            nsinT = const_p.tile([hd, tok], F32)
            nc.sync.dma_start(nsinT[:], nsinT_d[:])
            maskT = const_p.tile([s, rows], F32)
            nc.sync.dma_start(maskT[:], maskT_d[:])
            ident = const_p.tile([tok, tok], F32)
            from concourse.masks import make_identity
            make_identity(nc, ident[:])

            # ---- load hiddenT: [h, tok] -> sbuf [128, nch*tok] ----
            hT = hT_p.tile([128, nch * tok], BF16)
            nc.sync.dma_start(
                hT[:].rearrange("p (c t) -> p c t", c=nch),
                hiddenT_d.rearrange("(c p) t -> p c t", p=128),
            )

            # ---- projections: qT_ps[j] [128, tok], kT_ps [128, tok], v_ps [tok, 128] ----
            # q in token-major [tok, hq*hd] (single PSUM bank/group); k/v direct
            q_ps = ps.tile([tok, hq * hd], F32, tag="A")
            kT_ps = ps.tile([128, tok], F32, tag="B")
            v_ps = ps.tile([tok, 128], F32, tag="B")
            for c in range(nch):
                wq_t = wq_p.tile([128, hq * hd], BF16)
                nc.sync.dma_start(
                    wq_t[:], wq_d.rearrange("(c p) m -> c p m", p=128)[c]
                )
                wkv_t = wkv_p.tile([128, 2 * hd], BF16)
                nc.sync.dma_start(
                    wkv_t[:], wkv_d.rearrange("(c p) m -> c p m", p=128)[c]
                )
                rhs_h = hT[:, c * tok:(c + 1) * tok]
                fl = dict(start=(c == 0), stop=(c == nch - 1))
                nc.tensor.matmul(q_ps[:], rhs_h, wq_t[:], **fl)
                nc.tensor.matmul(kT_ps[:], wkv_t[:, 0:hd], rhs_h, **fl)
                nc.tensor.matmul(v_ps[:], rhs_h, wkv_t[:, hd:2 * hd], **fl)
            q_sb = qkv_p.tile([tok, hq * hd], F32, tag="qsb")
            nc.scalar.copy(q_sb[:], q_ps[:])

            # ---- RoPE -> qT_sb [128, (b,hq,s)], kT_new [128, (b,s)], v_new [tok, 128] ----
            half = hd // 2
            qT_sb = qkv_p.tile([128, b * rows], F32, tag="qT")
            kT_new = qkv_p.tile([128, tok], F32, tag="kTn")
            # per-batch fresh-v tiles at base partition 0 (PE wants base 0/32/64)
            v_new = [
                qkv_p.tile([s, hd], F32, tag=f"vnew{bb}", name=f"vnew{bb}")
                for bb in range(b)
            ]

            def rope(dst, src_ps):
                # dst = src*cos + rotate_half(src)*sin  (all [128, tok], (b,t) cols)
                t1 = rope_p.tile([128, tok], F32, tag="r1")
                nc.vector.tensor_mul(t1[:], src_ps[:], cosT[:])
                t2 = rope_p.tile([128, tok], F32, tag="r2")
                nc.vector.tensor_mul(
                    t2[0:half, :], src_ps[half:hd, :], nsinT[0:half, :]
                )
                nc.vector.tensor_mul(
                    t2[half:hd, :], src_ps[0:half, :], sinT[half:hd, :]
                )
                nc.vector.tensor_add(dst, t1[:], t2[:])
                return dst

            for j in range(hq):
                # transpose head j to [d, (b,t)], then rope-scatter to (b, j, t)
                qt_ps = ps.tile([hd, tok], F32, tag="sc", name=f"qtp{j}")
                nc.tensor.transpose(
                    qt_ps[:], q_sb[:, j * hd:(j + 1) * hd], ident[:]
                )
                dst = qT_sb[:].rearrange("p (bb j t) -> p bb j t", bb=b, j=hq)[:, :, j, :]
                rope(dst, qt_ps)
            rope(kT_new[:], kT_ps)
            v_sb = qkv_p.tile([tok, hd], F32, tag="vsb")
            nc.scalar.copy(v_sb[:], v_ps[:])
            for bb in range(b):
                nc.sync.dma_start(v_new[bb][:], v_sb[bb * s:(bb + 1) * s, :])

            qT_bf = qkv_p.tile([128, b * rows], BF16, tag="qTbf")
            nc.vector.tensor_copy(qT_bf[:], qT_sb[:])

            # ---- attention per batch ----
            # Scores are built 8 kpos-tiles at a time into ONE psum bank
            # (disjoint column ranges, one accumulation group) so exp / the
            # denominator reduce run 512 wide, 8x fewer cross-engine hops.
            GRP = 512 // rows               # kpos tiles per score group (8)
            attnT_sb = qkv_p.tile([128, hq * tok], BF16, tag="attnT")  # (h, b, t) cols
            for bb in range(b):
                qT_b = qT_bf[:, bb * rows:(bb + 1) * rows]  # [128, (h,t)] bf16
                qT_b32 = qT_sb[:, bb * rows:(bb + 1) * rows]
                attn_ps = ps.tile([128, rows], F32, tag="attn")
                acc = acc_p.tile([128, rows], F32, tag="acc")
                for hf in range(halves):
                    kt = kv_p.tile([128, kt_half * 128], BF16, tag="kt")
                    nc.sync.dma_start(
                        kt[:], kT_d[bb, :, hf * kt_half * 128:(hf + 1) * kt_half * 128]
                    )
                    vt = kv_p.tile([128, kt_half * hd], BF16, tag="vt")
                    nc.sync.dma_start(
                        vt[:],
                        v_d[bb, :, hf * kt_half * hd:(hf + 1) * kt_half * hd],
                    )
                    for g in range(kt_half // GRP):
                        sc_ps = ps.tile([128, GRP * rows], F32, tag="sc")
                        for u in range(GRP):
                            tt = g * GRP + u
                            nc.tensor.matmul(
                                sc_ps[:, u * rows:(u + 1) * rows],
                                kt[:, tt * 128:(tt + 1) * 128], qT_b,
                                start=(u == 0), stop=(u == GRP - 1),
                            )
                        ex = exp_p.tile([128, GRP * rows], BF16, tag="ex")
                        nc.scalar.activation(ex[:], sc_ps[:], AF.Exp)
                        red = acc if (hf == 0 and g == 0) else acc_p.tile(
                            [128, rows], F32, tag="red", name="red")
                        nc.vector.tensor_reduce(
                            red[:],
                            ex[:].rearrange("p (u q) -> p q u", u=GRP),
                            axis=mybir.AxisListType.X, op=mybir.AluOpType.add,
                        )
                        if red is not acc:
                            nc.vector.tensor_add(acc[:], acc[:], red[:])
                        for u in range(GRP):
                            tt = g * GRP + u
                            t = hf * kt_half + tt
                            nc.tensor.matmul(
                                attn_ps[:], vt[:, tt * hd:(tt + 1) * hd],
                                ex[:, u * rows:(u + 1) * rows],
                                start=(t == 0), stop=False, skip_group_check=True,
                            )
                # fresh keys (the only masked block)
                scn_ps = ps.tile([s, rows], F32, tag="sc")
                nc.tensor.matmul(
                    scn_ps[:], kT_new[:, bb * s:(bb + 1) * s], qT_b32,
                    start=True, stop=True,
                )
                exn = exp_p.tile([s, rows], F32, tag="exn")
                nc.scalar.activation(exn[:], scn_ps[:], AF.Exp)
                nc.vector.tensor_mul(exn[:], exn[:], maskT[:])
                nc.vector.tensor_add(acc[0:s, :], acc[0:s, :], exn[:])
                nc.tensor.matmul(
                    attn_ps[:], v_new[bb][:], exn[:],
                    start=False, stop=True, skip_group_check=True,
                )
                # denominator: reduce acc over partitions, broadcast reciprocal
                dsum_ps = ps.tile([1, rows], F32, tag="B")
                nc.tensor.matmul(dsum_ps[:], ones_col[:], acc[:], start=True, stop=True)
                rden = den_p.tile([1, rows], F32, tag="rden")
                nc.vector.reciprocal(rden[:], dsum_ps[:])
                bc_ps = ps.tile([128, rows], F32, tag="B")
                nc.tensor.matmul(bc_ps[:], ones_row[:], rden[:], start=True, stop=True)
                rdenb = den_p.tile([128, rows], F32, tag="rdenb")
                nc.scalar.copy(rdenb[:], bc_ps[:])
                # normalize + scatter (h,t) -> (h, b, t)
                dst = attnT_sb[:].rearrange("p (j bb t) -> p j bb t", j=hq, bb=b)[
                    :, :, bb, :
                ]
                nc.vector.tensor_mul(
                    dst,
                    attn_ps[:].rearrange("p (j t) -> p j t", j=hq),
                    rdenb[:].rearrange("p (j t) -> p j t", j=hq),
                )

            # ---- o_proj: out[tok, h] = sum_j attnT_j.T @ wo_j ----
            for nt in range(h // 512):
                o_ps = ps.tile([tok, 512], F32, tag="A")
                for j in range(hq):
                    wo_t = wo_p.tile([128, 512], BF16, tag="wo")
                    nc.sync.dma_start(
                        wo_t[:],
                        wo_d.rearrange("(j p) m -> j p m", p=128)[
                            j, :, nt * 512:(nt + 1) * 512
                        ],
                    )
                    nc.tensor.matmul(
                        o_ps[:], attnT_sb[:, j * tok:(j + 1) * tok], wo_t[:],
                        start=(j == 0), stop=(j == hq - 1),
                    )
                o_sb = wo_p.tile([tok, 512], F32, tag="osb", bufs=3)
                nc.scalar.copy(o_sb[:], o_ps[:])
                nc.sync.dma_start(out_d[:, nt * 512:(nt + 1) * 512], o_sb[:])

    nc.compile()
    return nc


_NC_CACHE = {}


def _get_nc(key=(B, S, H, HQ, HD, PAST)):
    if key not in _NC_CACHE:
        _NC_CACHE[key] = build_nc(*key)
    return _NC_CACHE[key]


def make_in_maps(hidden_states, k_cache, v_cache, Wq, Wk, Wv, Wo, position_ids):
    """Host-side shard + layout prep: one input dict per core."""
    hiddenT = np.ascontiguousarray(
        hidden_states.reshape(TOK, H).T.astype(np.float32)
    ).astype(ml_dtypes.bfloat16)
    # RoPE tables in [d, (b, t)] layout, duplicated freq block (half-split rope)
    inv_freq = (1.0 / (ROPE_BASE ** (np.arange(0, HD, 2, dtype=np.float64) / HD)))
    ang = position_ids.astype(np.float64).reshape(-1)[None, :] * np.concatenate(
        [inv_freq, inv_freq]
    )[:, None]                                           # [hd, tok]
    cosT = np.cos(ang).astype(np.float32)
    sinT = np.sin(ang).astype(np.float32)
    nsinT = (-sinT).copy()
    # mask over fresh keys: maskT[j, (h, t)] = 1 if j <= t (bottom-right causal)
    jj = np.arange(S)[:, None]
    tt = np.tile(np.arange(S)[None, :], (1, HQ)).reshape(1, ROWS)
    maskT = (jj <= tt).astype(np.float32)

    in_maps = []
    for c in range(NCORES):
        q0 = c * HQ * HD
        in_maps.append({
            "hiddenT": hiddenT,
            "wq": np.ascontiguousarray(
                (Wq[:, q0:q0 + HQ * HD] * SCALE).astype(np.float32)
            ).astype(ml_dtypes.bfloat16),
            "wkv": np.ascontiguousarray(
                np.concatenate(
                    [Wk[:, c * HD:(c + 1) * HD], Wv[:, c * HD:(c + 1) * HD]], axis=1
                ), dtype=np.float32).astype(ml_dtypes.bfloat16),
            "wo": np.ascontiguousarray(
                Wo[q0:q0 + HQ * HD, :].astype(np.float32)
            ).astype(ml_dtypes.bfloat16),
            "kT": np.ascontiguousarray(
                k_cache[:, :, c, :].transpose(0, 2, 1)).astype(ml_dtypes.bfloat16),
            # pre-permuted to the sbuf tile layout: v_r[b, p, tt*HD+d] =
            # v[b, tt*128+p, d] -> fully contiguous 8KB DMA rows
            "v": np.ascontiguousarray(
                v_cache[:, :, c, :].reshape(B, PAST // 128, 128, HD)
                .transpose(0, 2, 1, 3).reshape(B, 128, PAST)
            ).astype(ml_dtypes.bfloat16),
            "cosT": cosT, "sinT": sinT, "nsinT": nsinT, "maskT": maskT,
        })
    return in_maps


def kernel(hidden_states, k_cache, v_cache, Wq, Wk, Wv, Wo, position_ids):
    hidden_states = np.asarray(hidden_states)
    nc = _get_nc()
    in_maps = make_in_maps(
        np.asarray(hidden_states), np.asarray(k_cache), np.asarray(v_cache),
        np.asarray(Wq), np.asarray(Wk), np.asarray(Wv), np.asarray(Wo),
        np.asarray(position_ids),
    )
    res = run_bass_kernel_spmd(nc, in_maps, list(range(NCORES)))
    out = np.zeros((TOK, H), np.float32)
    for c in range(NCORES):
        out += res.results[c]["out_p"]
    return out.reshape(B, S, H)
